# revision 2
# baseline (speedup 1.0000x reference)
import sys
if '/opt/trn_rl_repo' not in sys.path:
    sys.path.insert(0, '/opt/trn_rl_repo')
import numpy as np
import ml_dtypes

import concourse.bass as bass
import concourse.bacc as bacc
import concourse.tile as tile
from concourse import mybir
from concourse.bass_utils import run_bass_kernel_spmd
from concourse.masks import make_identity

F32 = mybir.dt.float32
BF = mybir.dt.bfloat16
AF = mybir.ActivationFunctionType
MUL = mybir.AluOpType.mult
ADD = mybir.AluOpType.add
SUB = mybir.AluOpType.subtract
P = 128
D, H, DK, DV, NL = 768, 8, 64, 64, 2
B, LC, LQ, LK = 8, 512, 160, 512
DC = D // P      # 6 chunks of the 768 dim
CC = LC // P     # 4 chunks of the 512 token dim
QCH = [(0, 128), (128, 32)]   # (offset, size) chunks of LQ=160
SCALE = 0.125    # log_512(512)/sqrt(64)
EPS = 1e-6
NPBF = ml_dtypes.bfloat16

USE_SPART = True
USE_APPROX_RECIP = True
USE_GPS_BCAST = True
_CACHE = {}


def _build():
    nc = bacc.Bacc()
    dt = {}

    def din(name, shape, dtype=BF):
        dt[name] = nc.dram_tensor(name, list(shape), dtype, kind="ExternalInput")
        return dt[name]

    din('S_nat', (LC, D)); din('S_T', (D, LC))
    din('Q_nat', (LQ, D)); din('Q_T', (D, LQ))
    din('E_nat', (LQ, D)); din('E_T', (D, LQ))
    din('KE_T', (D, LK))
    din('vecs', (D, 4), F32)     # cols: w4C, w4Q, w4mlu, cqa_b
    din('cqa_WT', (4 * D, D))
    for l in range(NL):
        din(f'sWq{l}', (D, H * DK)); din(f'sWk{l}', (D, H * DK))
        din(f'sWv{l}', (D, H * DV)); din(f'sWfc{l}', (H * DV, D))
        din(f'cWq{l}', (D, H * DK)); din(f'cWk{l}', (2 * D, H * DK))
        din(f'cWv{l}', (2 * D, H * DV)); din(f'cWfc{l}', (H * DV, D))
        din(f'ln{l}', (D, 4), F32)   # cols: n1g, n1b, n2g, n2b
    out_t = nc.dram_tensor('out_t', [3 * D, LC], F32, kind="ExternalOutput")

    with tile.TileContext(nc) as tc:
        _emit(nc, tc, dt, out_t)
    nc.compile()
    return nc


def _emit(nc, tc, dt, out_t):
    from contextlib import ExitStack
    ctx = ExitStack()
    const = ctx.enter_context(tc.tile_pool(name="const", bufs=1))
    persist = ctx.enter_context(tc.tile_pool(name="persist", bufs=1))

    ident = const.tile([P, P], BF)
    make_identity(nc, ident)
    ones_row = const.tile([1, P], BF)
    nc.gpsimd.memset(ones_row, 1.0)
    ones_col = const.tile([P, 1], BF)
    nc.gpsimd.memset(ones_col, 1.0)
    eps_t = const.tile([1, 1], F32)
    nc.gpsimd.memset(eps_t, EPS)
    # vecs: [768,4] -> sbuf [128, 6, 4] (chunk dc at [:, dc, col]); fp32 + bf16 twin
    vecs = const.tile([P, DC, 4], F32)
    nc.sync.dma_start(out=vecs, in_=dt['vecs'].rearrange("(c p) v -> p c v", p=P))
    vecs_bf = const.tile([P, DC, 4], BF)
    nc.vector.tensor_copy(vecs_bf, vecs)
    lnv = []
    for l in range(NL):
        t = const.tile([P, DC, 4], F32, name=f"lnv{l}")
        nc.sync.dma_start(out=t, in_=dt[f'ln{l}'].rearrange("(c p) v -> p c v", p=P))
        lnv.append(t)

    # ---------------- phase 1: s2q twice ----------------
    s2q = tc.alloc_tile_pool(name="s2q", bufs=1)
    cqaw = tc.alloc_tile_pool(name="cqaw", bufs=1)
    ps = ctx.enter_context(tc.tile_pool(name="ps", bufs=1, space="PSUM"))

    cqa_WT = [cqaw.tile([P, D], BF, name=f"cqaWT{k}") for k in range(4 * DC)]
    for k in range(4 * DC):
        nc.sync.dma_start(out=cqa_WT[k], in_=dt['cqa_WT'][k * P:(k + 1) * P, :])

    S_nat = [s2q.tile([P, D], BF, name=f"Snat{c}") for c in range(CC)]
    for c in range(CC):
        nc.sync.dma_start(out=S_nat[c], in_=dt['S_nat'][c * P:(c + 1) * P, :])
    S_T = [s2q.tile([P, LC], BF, name=f"ST{d}") for d in range(DC)]
    for d in range(DC):
        nc.sync.dma_start(out=S_T[d], in_=dt['S_T'][d * P:(d + 1) * P, :])
    # cm_T = S_T * w4mlu (per-partition)
    cm_T = [s2q.tile([P, LC], BF, name=f"cmT{d}") for d in range(DC)]
    for d in range(DC):
        nc.vector.tensor_scalar_mul(cm_T[d], S_T[d], vecs[:, d, 2:3])
    # s0_row [1, LC]
    ps0 = ps.tile([1, LC], F32, tag="b", bufs=4)
    for d in range(DC):
        nc.tensor.matmul(ps0, vecs_bf[:, d, 0:1], S_T[d], start=(d == 0), stop=(d == DC - 1))
    s0_row = s2q.tile([1, LC], BF)
    nc.vector.tensor_copy(s0_row, ps0)
    # cqa S_T contribution (shared between the q and e calls): Spart[mc] [P, LC] fp32
    # att held in SBUF (bf16) for phase 2 kv
    att_T = [persist.tile([P, LC], BF, name=f"attT{i}") for i in range(2 * DC)]

    def s2q_call(tag, QN, QT, row0):
        po = tc.alloc_tile_pool(name=f"s2qt_{tag}", bufs=1)
        Qn, Qt = [], []
        for qi, (qo, qs) in enumerate(QCH):
            t = po.tile([P, D], BF, name=f"Qn{tag}{qi}")
            nc.sync.dma_start(out=t[:qs], in_=QN[qo:qo + qs, :])
            Qn.append(t)
        for d in range(DC):
            t = po.tile([P, LQ], BF, name=f"Qt{tag}{d}")
            nc.sync.dma_start(out=t, in_=QT[d * P:(d + 1) * P, :])
            Qt.append(t)
        # s1 [LQ,1] fp32 (exp bias)
        s1 = []
        for qi, (qo, qs) in enumerate(QCH):
            pq = ps.tile([P, 1], F32, tag="b", bufs=4)
            for d in range(DC):
                nc.tensor.matmul(pq[:qs], Qt[d][:, qo:qo + qs], vecs_bf[:, d, 1:2],
                                 start=(d == 0), stop=(d == DC - 1))
            t = po.tile([P, 1], F32, name=f"s1{tag}{qi}")
            nc.vector.tensor_copy(t[:qs], pq[:qs])
            s1.append(t)
        # score_T + exp -> e_t (bf16), row sums -> st (fp32); etn = e_t/rowsum
        e_t, etn = [], []
        for qi, (qo, qs) in enumerate(QCH):
            psc_t = ps.tile([P, LC], F32, tag="a", bufs=4)
            for d in range(DC):
                nc.tensor.matmul(psc_t[:qs], Qt[d][:, qo:qo + qs], cm_T[d],
                                 start=(d == 0), stop=False)
            nc.tensor.matmul(psc_t[:qs], ones_row[:1, :qs], s0_row,
                             start=False, stop=True)
            et = po.tile([P, LC], BF, name=f"et{tag}{qi}")
            st = po.tile([P, 1], F32, name=f"st{tag}{qi}")
            nc.scalar.activation(et[:qs], psc_t[:qs], AF.Exp, bias=s1[qi][:qs],
                                 scale=1.0, accum_out=st[:qs])
            rt = po.tile([P, 1], F32, name=f"rt{tag}{qi}")
            if USE_APPROX_RECIP:
                nc.vector.reciprocal_approx_fast(out=rt[:qs], in_=st[:qs])
            else:
                nc.vector.reciprocal(rt[:qs], st[:qs])
            en = po.tile([P, LC], BF, name=f"etn{tag}{qi}")
            nc.vector.tensor_scalar_mul(en[:qs], et[:qs], rt[:qs])
            e_t.append(et); etn.append(en)
        # col sums over q (partitions) -> rc_row; P_T = e_t * bcast(rc_row)
        psr = ps.tile([1, LC], F32, tag="b", bufs=4)
        for qi, (qo, qs) in enumerate(QCH):
            nc.tensor.matmul(psr, ones_col[:qs, :1], e_t[qi][:qs],
                             start=(qi == 0), stop=(qi == 1))
        rc_row = po.tile([1, LC], F32, name=f"rc{tag}")
        if USE_APPROX_RECIP:
            rcs = po.tile([1, LC], F32, name=f"rcs{tag}")
            nc.scalar.copy(rcs, psr)
            nc.vector.reciprocal_approx_fast(out=rc_row, in_=rcs)
        else:
            nc.vector.reciprocal(rc_row, psr)
        pbs = po.tile([P, LC], F32, name=f"pbs{tag}")
        if USE_GPS_BCAST:
            nc.gpsimd.partition_broadcast(pbs, rc_row)
        else:
            rc_bf = po.tile([1, LC], BF, name=f"rcbf{tag}")
            nc.vector.tensor_copy(rc_bf, rc_row)
            pbp = ps.tile([P, LC], F32, tag="a", bufs=4)
            nc.tensor.matmul(pbp, ones_row, rc_bf)
            nc.vector.tensor_copy(pbs, pbp)
        P_T = []
        for qi, (qo, qs) in enumerate(QCH):
            pt = po.tile([P, LC], BF, name=f"PT{tag}{qi}")
            nc.vector.tensor_tensor(pt[:qs], e_t[qi][:qs], pbs[:qs], op=MUL)
            P_T.append(pt)
        # etn_T [LC, LQ]: transpose etn
        etn_T = [po.tile([P, LQ], BF, name=f"etnT{tag}{c}") for c in range(CC)]
        for c in range(CC):
            for qi, (qo, qs) in enumerate(QCH):
                pt = ps.tile([P, P], BF, tag="b", bufs=4)
                nc.tensor.transpose(pt[:, :qs], etn[qi][:qs, c * P:(c + 1) * P],
                                    ident[:qs, :qs])
                nc.vector.tensor_copy(etn_T[c][:, qo:qo + qs], pt[:, :qs])
        # tmp [LQ, D]
        tmp = []
        for qi, (qo, qs) in enumerate(QCH):
            t = po.tile([P, D], BF, name=f"tmp{tag}{qi}")
            for n in range(2):
                pm = ps.tile([P, 384], F32, tag="a", bufs=4)
                for c in range(CC):
                    nc.tensor.matmul(pm[:qs], etn_T[c][:, qo:qo + qs],
                                     S_nat[c][:, n * 384:(n + 1) * 384],
                                     start=(c == 0), stop=(c == CC - 1))
                nc.vector.tensor_copy(t[:qs, n * 384:(n + 1) * 384], pm[:qs])
            tmp.append(t)
        # c2q_T, m1, m2 (the X4^T blocks beyond S_T and c2q_T itself)
        c2q_T = [po.tile([P, LC], BF, name=f"c2qT{tag}{d}") for d in range(DC)]
        m1 = [po.tile([P, LC], BF, name=f"m1{tag}{d}") for d in range(DC)]
        m2 = [po.tile([P, LC], BF, name=f"m2{tag}{d}") for d in range(DC)]
        for d in range(DC):
            pc = ps.tile([P, LC], F32, tag="a", bufs=4)
            for qi, (qo, qs) in enumerate(QCH):
                nc.tensor.matmul(pc, Qn[qi][:qs, d * P:(d + 1) * P], P_T[qi][:qs],
                                 start=(qi == 0), stop=(qi == 1))
            nc.vector.tensor_copy(c2q_T[d], pc)
            nc.vector.tensor_tensor(m1[d], c2q_T[d], S_T[d], op=MUL)
            pq2 = ps.tile([P, LC], F32, tag="a", bufs=4)
            for qi, (qo, qs) in enumerate(QCH):
                nc.tensor.matmul(pq2, tmp[qi][:qs, d * P:(d + 1) * P], P_T[qi][:qs],
                                 start=(qi == 0), stop=(qi == 1))
            nc.vector.tensor_tensor(m2[d], pq2, S_T[d], op=MUL)
        # cqa: out^T[dout, c] = Spart (preloaded) + remaining 18 k-blocks
        xblocks = S_T + c2q_T + m1 + m2
        for mc in range(DC):
            pco = ps.tile([P, LC], F32, tag="a", bufs=4)
            if USE_SPART:
                nc.scalar.copy(pco, Spart[mc])
                for k in range(DC, 4 * DC):
                    nc.tensor.matmul(pco, cqa_WT[k][:, mc * P:(mc + 1) * P],
                                     xblocks[k], start=False, stop=(k == 4 * DC - 1),
                                     skip_group_check=True)
            else:
                for k in range(4 * DC):
                    nc.tensor.matmul(pco, cqa_WT[k][:, mc * P:(mc + 1) * P],
                                     xblocks[k], start=(k == 0), stop=(k == 4 * DC - 1))
            ob = po.tile([P, LC], F32, name=f"ob{tag}{mc}", tag="attb", bufs=2)
            nc.scalar.activation(ob, pco, AF.Identity,
                                 bias=vecs[:, mc, 3:4], scale=1.0)
            nc.sync.dma_start(out=out_t[(row0 + mc) * P:(row0 + mc + 1) * P, :],
                              in_=ob)
            nc.vector.tensor_copy(att_T[row0 + mc], ob)
        return po

    po_q = s2q_call("q", dt['Q_nat'], dt['Q_T'], 0)
    po_q.release()
    po_e = s2q_call("e", dt['E_nat'], dt['E_T'], DC)
    po_e.release()
    cqaw.release(); s2q.release()

    # ---------------- phase 2: knowledge attention stack ----------------
    mp = ctx.enter_context(tc.tile_pool(name="mp", bufs=1))
    wls[1] = alloc_wl(1)
    ke_T = [mp.tile([P, LK], BF, name=f"keT{d}", tag=f"ke{d}", bufs=2) for d in range(DC)]
    for d in range(DC):
        nc.sync.dma_start(out=ke_T[d], in_=dt['KE_T'][d * P:(d + 1) * P, :])

    def mha_ln(x_T, kv_T, wq, wk, wv, wfc, g_ap, b_ap, tag, out_f32=False):
        """x_T: 6 [P,LK] bf16 query-side tiles; kv_T: list of [P,LK] bf16 tiles.
        returns new 6 [P,LK] tiles = LN(fc(attn) + x_T) (bf16, or f32 if out_f32)."""
        nkv = len(kv_T)
        wp = tc.alloc_tile_pool(name=f"wp{tag}", bufs=1)
        # --- projections (streamed weights) ---
        def proj(w_dram, rhs_tiles, nk, out_name, tagbase):
            outs = [mp.tile([P, LK], BF, name=f"{out_name}{m}", tag=f"{tagbase}{m}",
                            bufs=1) for m in range(4)]
            pss = [ps.tile([P, LK], F32, name=f"pss{m}", tag="a", bufs=4) for m in range(4)]
            for k in range(nk):
                wt = wp.tile([P, H * DK], BF, name=f"w{out_name}{k}",
                             tag=f"w{out_name}", bufs=3)
                nc.sync.dma_start(out=wt, in_=w_dram[k * P:(k + 1) * P, :])
                src = rhs_tiles[k]
                for m in range(4):
                    nc.tensor.matmul(pss[m], wt[:, m * P:(m + 1) * P], src,
                                     start=(k == 0), stop=(k == nk - 1))
            for m in range(4):
                nc.vector.tensor_copy(outs[m], pss[m])
            return outs

        q_T = proj(wq, x_T, DC, f"q{tag}", "qT")
        k_T = proj(wk, kv_T, nkv, f"k{tag}", "kT")
        # v in natural layout + ones col: v_aug [LK, 8, 65]
        v_aug = [mp.tile([P, H, DV + 1], BF, name=f"va{tag}{c}", tag=f"va{c}", bufs=1)
                 for c in range(CC)]
        pvs = [ps.tile([P, H * DV], F32, name=f"pvs{m}", tag="a", bufs=4) for m in range(4)]
        for k in range(nkv):
            wt = wp.tile([P, H * DV], BF, name=f"wv{tag}{k}", tag="wv", bufs=3)
            nc.sync.dma_start(out=wt, in_=wv[k * P:(k + 1) * P, :])
            for c in range(CC):
                nc.tensor.matmul(pvs[c], kv_T[k][:, c * P:(c + 1) * P], wt,
                                 start=(k == 0), stop=(k == nkv - 1))
        for c in range(CC):
            nc.vector.tensor_copy(v_aug[c][:, :, 0:DV],
                                  pvs[c].rearrange("p (h d) -> p h d", h=H))
            nc.gpsimd.memset(v_aug[c][:, :, DV:DV + 1], 1.0)
        # --- attention, heads in groups of 4 (batched denominators) ---
        out_T = [mp.tile([P, LK], BF, name=f"o{tag}{m}", tag=f"oT{m}", bufs=1)
                 for m in range(4)]
        for g in range(2):
            povs = []
            for hh in range(4):
                h = g * 4 + hh
                t, o = h // 2, (h % 2) * DK
                e_sb = []
                for c in range(CC):
                    pa = ps.tile([P, LK], F32, tag="a", bufs=4)
                    nc.tensor.matmul(pa, k_T[t][o:o + DK, c * P:(c + 1) * P],
                                     q_T[t][o:o + DK, :], start=True, stop=True)
                    es = mp.tile([P, LK], BF, name=f"es{tag}{h}{c}", tag="es", bufs=8)
                    nc.scalar.activation(es, pa, AF.Exp, scale=SCALE)
                    e_sb.append(es)
                pov = ps.tile([DV + 1, LK], F32, tag="b", bufs=4)
                for c in range(CC):
                    nc.tensor.matmul(pov, v_aug[c][:, h, :], e_sb[c],
                                     start=(c == 0), stop=(c == CC - 1))
                povs.append(pov)
            for hh in range(4):
                h = g * 4 + hh
                t, o = h // 2, (h % 2) * DK
                rr = mp.tile([1, LK], F32, name=f"rr{tag}{h}", tag="rr", bufs=2)
                if USE_APPROX_RECIP:
                    rrs = mp.tile([1, LK], F32, name=f"rrs{tag}{h}", tag="rrs", bufs=2)
                    nc.scalar.copy(rrs, povs[hh][DV:DV + 1, :])
                    nc.vector.reciprocal_approx_fast(out=rr, in_=rrs)
                else:
                    nc.vector.reciprocal(rr, povs[hh][DV:DV + 1, :])
                pbc = mp.tile([DV, LK], F32, name=f"pbc{tag}{h}", tag="pbc", bufs=2)
                if USE_GPS_BCAST:
                    nc.gpsimd.partition_broadcast(pbc, rr)
                else:
                    rr_bf = mp.tile([1, LK], BF, name=f"rrbf{tag}{h}", tag="rrbf", bufs=4)
                    nc.vector.tensor_copy(rr_bf, rr)
                    pbq = ps.tile([DV, LK], F32, tag="a", bufs=4)
                    nc.tensor.matmul(pbq, ones_row[:1, :DV], rr_bf)
                    nc.vector.tensor_copy(pbc, pbq)
                nc.vector.tensor_tensor(out_T[t][o:o + DK, :], povs[hh][:DV, :],
                                        pbc, op=MUL)
        # --- fc + residual + LN ---
        wf = [wp.tile([P, D], BF, name=f"wf{tag}{k}", tag="wf", bufs=4)
              for k in range(4)]
        for k in range(4):
            nc.sync.dma_start(out=wf[k], in_=wfc[k * P:(k + 1) * P, :])
        x1 = [mp.tile([P, LK], BF, name=f"x1{tag}{d}", tag=f"x1{d}", bufs=1)
              for d in range(DC)]
        for d in range(DC):
            pf = ps.tile([P, LK], F32, tag="a", bufs=4)
            for k in range(4):
                nc.tensor.matmul(pf, wf[k][:, d * P:(d + 1) * P], out_T[k],
                                 start=(k == 0), stop=(k == 3))
            nc.vector.tensor_tensor(x1[d], pf, x_T[d], op=ADD)
        # LN stats via ones-matmul over partitions
        ps_s = ps.tile([1, LK], F32, tag="b", bufs=4)
        ps_q = ps.tile([1, LK], F32, tag="b", bufs=4)
        sqs = [mp.tile([P, LK], BF, name=f"sq{tag}{d}", tag="sq", bufs=3)
               for d in range(DC)]
        for d in range(DC):
            nc.vector.tensor_tensor(sqs[d], x1[d], x1[d], op=MUL)
        for d in range(DC):
            nc.tensor.matmul(ps_s, ones_col, x1[d], start=(d == 0), stop=(d == DC - 1))
        for d in range(DC):
            nc.tensor.matmul(ps_q, ones_col, sqs[d], start=(d == 0), stop=(d == DC - 1))
        mu = mp.tile([1, LK], F32, name=f"mu{tag}", tag="mu", bufs=1)
        nc.scalar.activation(mu, ps_s, AF.Copy, bias=0.0, scale=1.0 / D)
        msq = mp.tile([1, LK], F32, name=f"msq{tag}", tag="msq", bufs=1)
        nc.scalar.activation(msq, ps_q, AF.Copy, bias=0.0, scale=1.0 / D)
        var = mp.tile([1, LK], F32, name=f"var{tag}", tag="var", bufs=1)
        nc.vector.tensor_tensor(var, mu, mu, op=MUL)
        nc.vector.tensor_tensor(var, msq, var, op=SUB)
        std = mp.tile([1, LK], F32, name=f"std{tag}", tag="std", bufs=1)
        nc.scalar.activation(std, var, AF.Sqrt, bias=eps_t, scale=1.0)
        warm2 = mp.tile([1, 1], F32, name=f"warm2{tag}", tag="warm", bufs=2)
        nc.scalar.activation(warm2, eps_t, AF.Exp, bias=eps_t, scale=1.0)
        rstd = mp.tile([1, LK], F32, name=f"rstd{tag}", tag="rstd", bufs=1)
        if USE_APPROX_RECIP:
            nc.vector.reciprocal_approx_fast(out=rstd, in_=std)
        else:
            nc.vector.reciprocal(rstd, std)
        c2 = mp.tile([1, LK], F32, name=f"c2{tag}", tag="c2", bufs=1)
        nc.vector.tensor_tensor(c2, mu, rstd, op=MUL)
        pA = mp.tile([P, LK], F32, name=f"pA{tag}", tag="pA", bufs=1)
        pC = mp.tile([P, LK], F32, name=f"pC{tag}", tag="pC", bufs=1)
        if USE_GPS_BCAST:
            nc.gpsimd.partition_broadcast(pA, rstd)
            nc.gpsimd.partition_broadcast(pC, c2)
        else:
            rstd_bf = mp.tile([1, LK], BF, name=f"rstdbf{tag}", tag="rstdbf", bufs=2)
            nc.vector.tensor_copy(rstd_bf, rstd)
            c2_bf = mp.tile([1, LK], BF, name=f"c2bf{tag}", tag="c2bf", bufs=2)
            nc.vector.tensor_copy(c2_bf, c2)
            pAp = ps.tile([P, LK], F32, tag="a", bufs=4)
            nc.tensor.matmul(pAp, ones_row, rstd_bf)
            nc.vector.tensor_copy(pA, pAp)
            pCp = ps.tile([P, LK], F32, tag="a", bufs=4)
            nc.tensor.matmul(pCp, ones_row, c2_bf)
            nc.vector.tensor_copy(pC, pCp)
        ydt = F32 if out_f32 else BF
        y = [mp.tile([P, LK], ydt, name=f"y{tag}{d}", tag=f"y{tag[0]}{d}", bufs=1)
             for d in range(DC)]
        for d in range(DC):
            nc.vector.tensor_tensor(y[d], x1[d], pA, op=MUL)
            nc.vector.tensor_tensor(y[d], y[d], pC, op=SUB)
            nc.vector.tensor_scalar(y[d], y[d], g_ap[d], b_ap[d],
                                    op0=MUL, op1=ADD)
        wp.release()
        return y

    cur = ke_T
    for l in range(NL):
        g1 = [lnv[l][:, d, 0:1] for d in range(DC)]
        b1 = [lnv[l][:, d, 1:2] for d in range(DC)]
        g2 = [lnv[l][:, d, 2:3] for d in range(DC)]
        b2 = [lnv[l][:, d, 3:4] for d in range(DC)]
        so = mha_ln(cur, cur, dt[f'sWq{l}'], dt[f'sWk{l}'], dt[f'sWv{l}'],
                    dt[f'sWfc{l}'], g1, b1, f"s{l}")
        cur = mha_ln(so, att_T, dt[f'cWq{l}'], dt[f'cWk{l}'], dt[f'cWv{l}'],
                     dt[f'cWfc{l}'], g2, b2, f"c{l}", out_f32=(l == NL - 1))
    for d in range(DC):
        nc.sync.dma_start(out=out_t[(2 * DC + d) * P:(2 * DC + d + 1) * P, :],
                          in_=cur[d])
    wls[1][0].release()
    ctx.close()


def kernel(**inputs):
    if 'nc' not in _CACHE:
        _CACHE['nc'] = _build()
    nc = _CACHE['nc']
    f = lambda x: np.ascontiguousarray(np.asarray(x), dtype=np.float32)
    bf = lambda x: np.ascontiguousarray(np.asarray(x, dtype=np.float32).astype(NPBF))
    bfT = lambda x: np.ascontiguousarray(np.asarray(x, dtype=np.float32).T.astype(NPBF))
    seq = f(inputs['sequences']); qry = f(inputs['query']); evd = f(inputs['evidence'])
    ke = f(inputs['knowledge_embed'])
    vecs = np.stack([f(inputs['w4C'])[:, 0], f(inputs['w4Q'])[:, 0],
                     f(inputs['w4mlu'])[0, 0, :], f(inputs['cqa_b'])], axis=1)
    vecs = np.ascontiguousarray(vecs)
    cqa_WT = bfT(inputs['cqa_W'])
    lwb = {}
    for l in range(NL):
        lwb[f'sWq{l}'] = bf(inputs['L_sWq'][l]); lwb[f'sWk{l}'] = bf(inputs['L_sWk'][l])
        lwb[f'sWv{l}'] = bf(inputs['L_sWv'][l]); lwb[f'sWfc{l}'] = bf(inputs['L_sWfc'][l])
        lwb[f'cWq{l}'] = bf(inputs['L_cWq'][l]); lwb[f'cWk{l}'] = bf(inputs['L_cWk'][l])
        lwb[f'cWv{l}'] = bf(inputs['L_cWv'][l]); lwb[f'cWfc{l}'] = bf(inputs['L_cWfc'][l])
        lwb[f'ln{l}'] = np.ascontiguousarray(np.stack(
            [f(inputs['L_n1g'][l]), f(inputs['L_n1b'][l]),
             f(inputs['L_n2g'][l]), f(inputs['L_n2b'][l])], axis=1))
    in_maps = []
    for b in range(B):
        m = {
            'S_nat': bf(seq[b]), 'S_T': bfT(seq[b]),
            'Q_nat': bf(qry[b]), 'Q_T': bfT(qry[b]),
            'E_nat': bf(evd[b]), 'E_T': bfT(evd[b]),
            'KE_T': bfT(ke[b]),
            'vecs': vecs, 'cqa_WT': cqa_WT,
        }
        m.update(lwb)
        in_maps.append(m)
    _CACHE['last_in_maps'] = in_maps
    res = run_bass_kernel_spmd(nc, in_maps, core_ids=list(range(B)))
    _CACHE['last_results'] = res
    outs = np.stack([r['out_t'] for r in res.results])          # [B, 2304, 512]
    out = np.concatenate([seq, outs.transpose(0, 2, 1)], axis=-1)
    return out


# revision 4
# speedup vs baseline: 1.0882x; 1.0882x over previous
import sys
if '/opt/trn_rl_repo' not in sys.path:
    sys.path.insert(0, '/opt/trn_rl_repo')
import numpy as np
import ml_dtypes

import concourse.bass as bass
import concourse.bacc as bacc
import concourse.tile as tile
from concourse import mybir
from concourse.bass_utils import run_bass_kernel_spmd
from concourse.masks import make_identity

F32 = mybir.dt.float32
BF = mybir.dt.bfloat16
AF = mybir.ActivationFunctionType
MUL = mybir.AluOpType.mult
ADD = mybir.AluOpType.add
SUB = mybir.AluOpType.subtract
P = 128
D, H, DK, DV, NL = 768, 8, 64, 64, 2
B, LC, LQ, LK = 8, 512, 160, 512
DC = D // P      # 6 chunks of the 768 dim
CC = LC // P     # 4 chunks of the 512 token dim
QCH = [(0, 128), (128, 32)]   # (offset, size) chunks of LQ=160
SCALE = 0.125    # log_512(512)/sqrt(64)
EPS = 1e-6
NPBF = ml_dtypes.bfloat16

USE_SPART = True
USE_APPROX_RECIP = True
USE_GPS_BCAST = True
_CACHE = {}


def _build():
    nc = bacc.Bacc()
    dt = {}

    def din(name, shape, dtype=BF):
        dt[name] = nc.dram_tensor(name, list(shape), dtype, kind="ExternalInput")
        return dt[name]

    # all big tensors pre-tiled on host to [128, chunks*width] (partition-contiguous)
    din('S_nat', (P, CC * D)); din('S_T', (P, DC * LC))
    din('Q_nat', (P, 2 * D)); din('Q_T', (P, DC * LQ))
    din('E_nat', (P, 2 * D)); din('E_T', (P, DC * LQ))
    din('KE_T', (P, DC * LK))
    din('vecs', (P, DC * 4), F32)    # cols: w4C, w4Q, w4mlu, cqa_b
    din('cqa_WT', (P, 4 * DC * D))
    for l in range(NL):
        din(f'sWq{l}', (P, DC * H * DK)); din(f'sWk{l}', (P, DC * H * DK))
        din(f'sWv{l}', (P, DC * H * DV)); din(f'sWfc{l}', (P, 4 * D))
        din(f'cWq{l}', (P, DC * H * DK)); din(f'cWk{l}', (P, 2 * DC * H * DK))
        din(f'cWv{l}', (P, 2 * DC * H * DV)); din(f'cWfc{l}', (P, 4 * D))
        din(f'ln{l}', (P, DC * 4), F32)   # cols: n1g, n1b, n2g, n2b
    out_t = nc.dram_tensor('out_t', [3 * D, LC], BF, kind="ExternalOutput")

    with tile.TileContext(nc) as tc:
        _emit(nc, tc, dt, out_t)
    nc.compile()
    return nc


def _emit(nc, tc, dt, out_t):
    from contextlib import ExitStack
    ctx = ExitStack()
    const = ctx.enter_context(tc.tile_pool(name="const", bufs=1))
    persist = ctx.enter_context(tc.tile_pool(name="persist", bufs=1))

    ident = const.tile([P, P], BF)
    make_identity(nc, ident)
    ones_row = const.tile([1, P], BF)
    nc.gpsimd.memset(ones_row, 1.0)
    ones_col = const.tile([P, 1], BF)
    nc.gpsimd.memset(ones_col, 1.0)
    eps_t = const.tile([1, 1], F32)
    nc.gpsimd.memset(eps_t, EPS)
    # vecs: [768,4] -> sbuf [128, 6, 4] (chunk dc at [:, dc, col]); fp32 + bf16 twin
    vecs = const.tile([P, DC, 4], F32)
    nc.sync.dma_start(out=vecs, in_=dt['vecs'][:, :])
    vecs_bf = const.tile([P, DC, 4], BF)
    nc.vector.tensor_copy(vecs_bf, vecs)
    lnv = []
    for l in range(NL):
        t = const.tile([P, DC, 4], F32, name=f"lnv{l}")
        nc.sync.dma_start(out=t, in_=dt[f'ln{l}'].rearrange("(c p) v -> p c v", p=P))
        lnv.append(t)

    # ---------------- phase 1: s2q twice ----------------
    s2q = tc.alloc_tile_pool(name="s2q", bufs=1)
    cqaw = tc.alloc_tile_pool(name="cqaw", bufs=1)
    ps = ctx.enter_context(tc.tile_pool(name="ps", bufs=1, space="PSUM"))

    cqa_WT = [cqaw.tile([P, D], BF, name=f"cqaWT{k}") for k in range(4 * DC)]
    for k in range(4 * DC):
        nc.sync.dma_start(out=cqa_WT[k], in_=dt['cqa_WT'][k * P:(k + 1) * P, :])

    S_nat = [s2q.tile([P, D], BF, name=f"Snat{c}") for c in range(CC)]
    for c in range(CC):
        nc.sync.dma_start(out=S_nat[c], in_=dt['S_nat'][c * P:(c + 1) * P, :])
    S_T = [s2q.tile([P, LC], BF, name=f"ST{d}") for d in range(DC)]
    for d in range(DC):
        nc.sync.dma_start(out=S_T[d], in_=dt['S_T'][d * P:(d + 1) * P, :])
    # cm_T = S_T * w4mlu (per-partition)
    cm_T = [s2q.tile([P, LC], BF, name=f"cmT{d}") for d in range(DC)]
    for d in range(DC):
        nc.vector.tensor_scalar_mul(cm_T[d], S_T[d], vecs[:, d, 2:3])
    # s0_row [1, LC]
    ps0 = ps.tile([1, LC], F32, tag="b", bufs=4)
    for d in range(DC):
        nc.tensor.matmul(ps0, vecs_bf[:, d, 0:1], S_T[d], start=(d == 0), stop=(d == DC - 1))
    s0_row = s2q.tile([1, LC], BF)
    nc.vector.tensor_copy(s0_row, ps0)
    # cqa S_T contribution (shared between the q and e calls): Spart[mc] [P, LC] fp32
    # att held in SBUF (bf16) for phase 2 kv
    att_T = [persist.tile([P, LC], BF, name=f"attT{i}") for i in range(2 * DC)]

    def s2q_call(tag, QN, QT, row0):
        po = tc.alloc_tile_pool(name=f"s2qt_{tag}", bufs=1)
        Qn, Qt = [], []
        for qi, (qo, qs) in enumerate(QCH):
            t = po.tile([P, D], BF, name=f"Qn{tag}{qi}")
            nc.sync.dma_start(out=t[:qs], in_=QN[qo:qo + qs, :])
            Qn.append(t)
        for d in range(DC):
            t = po.tile([P, LQ], BF, name=f"Qt{tag}{d}")
            nc.sync.dma_start(out=t, in_=QT[d * P:(d + 1) * P, :])
            Qt.append(t)
        # s1 [LQ,1] fp32 (exp bias)
        s1 = []
        for qi, (qo, qs) in enumerate(QCH):
            pq = ps.tile([P, 1], F32, tag="b", bufs=4)
            for d in range(DC):
                nc.tensor.matmul(pq[:qs], Qt[d][:, qo:qo + qs], vecs_bf[:, d, 1:2],
                                 start=(d == 0), stop=(d == DC - 1))
            t = po.tile([P, 1], F32, name=f"s1{tag}{qi}")
            nc.vector.tensor_copy(t[:qs], pq[:qs])
            s1.append(t)
        # score_T + exp -> e_t (bf16), row sums -> st (fp32); etn = e_t/rowsum
        e_t, etn = [], []
        for qi, (qo, qs) in enumerate(QCH):
            psc_t = ps.tile([P, LC], F32, tag="a", bufs=4)
            for d in range(DC):
                nc.tensor.matmul(psc_t[:qs], Qt[d][:, qo:qo + qs], cm_T[d],
                                 start=(d == 0), stop=False)
            nc.tensor.matmul(psc_t[:qs], ones_row[:1, :qs], s0_row,
                             start=False, stop=True)
            et = po.tile([P, LC], BF, name=f"et{tag}{qi}")
            st = po.tile([P, 1], F32, name=f"st{tag}{qi}")
            nc.scalar.activation(et[:qs], psc_t[:qs], AF.Exp, bias=s1[qi][:qs],
                                 scale=1.0, accum_out=st[:qs])
            rt = po.tile([P, 1], F32, name=f"rt{tag}{qi}")
            if USE_APPROX_RECIP:
                nc.vector.reciprocal_approx_fast(out=rt[:qs], in_=st[:qs])
            else:
                nc.vector.reciprocal(rt[:qs], st[:qs])
            en = po.tile([P, LC], BF, name=f"etn{tag}{qi}")
            nc.vector.tensor_scalar_mul(en[:qs], et[:qs], rt[:qs])
            e_t.append(et); etn.append(en)
        # col sums over q (partitions) -> rc_row; P_T = e_t * bcast(rc_row)
        psr = ps.tile([1, LC], F32, tag="b", bufs=4)
        for qi, (qo, qs) in enumerate(QCH):
            nc.tensor.matmul(psr, ones_col[:qs, :1], e_t[qi][:qs],
                             start=(qi == 0), stop=(qi == 1))
        rc_row = po.tile([1, LC], F32, name=f"rc{tag}")
        if USE_APPROX_RECIP:
            rcs = po.tile([1, LC], F32, name=f"rcs{tag}")
            nc.scalar.copy(rcs, psr)
            nc.vector.reciprocal_approx_fast(out=rc_row, in_=rcs)
        else:
            nc.vector.reciprocal(rc_row, psr)
        pbs = po.tile([P, LC], F32, name=f"pbs{tag}")
        if USE_GPS_BCAST:
            nc.gpsimd.partition_broadcast(pbs, rc_row)
        else:
            rc_bf = po.tile([1, LC], BF, name=f"rcbf{tag}")
            nc.vector.tensor_copy(rc_bf, rc_row)
            pbp = ps.tile([P, LC], F32, tag="a", bufs=4)
            nc.tensor.matmul(pbp, ones_row, rc_bf)
            nc.vector.tensor_copy(pbs, pbp)
        P_T = []
        for qi, (qo, qs) in enumerate(QCH):
            pt = po.tile([P, LC], BF, name=f"PT{tag}{qi}")
            nc.vector.tensor_tensor(pt[:qs], e_t[qi][:qs], pbs[:qs], op=MUL)
            P_T.append(pt)
        # etn_T [LC, LQ]: transpose etn
        etn_T = [po.tile([P, LQ], BF, name=f"etnT{tag}{c}") for c in range(CC)]
        for c in range(CC):
            for qi, (qo, qs) in enumerate(QCH):
                pt = ps.tile([P, P], BF, tag="b", bufs=4)
                nc.tensor.transpose(pt[:, :qs], etn[qi][:qs, c * P:(c + 1) * P],
                                    ident[:qs, :qs])
                nc.vector.tensor_copy(etn_T[c][:, qo:qo + qs], pt[:, :qs])
        # tmp [LQ, D]
        tmp = []
        for qi, (qo, qs) in enumerate(QCH):
            t = po.tile([P, D], BF, name=f"tmp{tag}{qi}")
            for n in range(2):
                pm = ps.tile([P, 384], F32, tag="a", bufs=4)
                for c in range(CC):
                    nc.tensor.matmul(pm[:qs], etn_T[c][:, qo:qo + qs],
                                     S_nat[c][:, n * 384:(n + 1) * 384],
                                     start=(c == 0), stop=(c == CC - 1))
                nc.vector.tensor_copy(t[:qs, n * 384:(n + 1) * 384], pm[:qs])
            tmp.append(t)
        # c2q_T, m1, m2 (the X4^T blocks beyond S_T and c2q_T itself)
        c2q_T = [po.tile([P, LC], BF, name=f"c2qT{tag}{d}") for d in range(DC)]
        m1 = [po.tile([P, LC], BF, name=f"m1{tag}{d}") for d in range(DC)]
        m2 = [po.tile([P, LC], BF, name=f"m2{tag}{d}") for d in range(DC)]
        for d in range(DC):
            pc = ps.tile([P, LC], F32, tag="a", bufs=4)
            for qi, (qo, qs) in enumerate(QCH):
                nc.tensor.matmul(pc, Qn[qi][:qs, d * P:(d + 1) * P], P_T[qi][:qs],
                                 start=(qi == 0), stop=(qi == 1))
            nc.vector.tensor_copy(c2q_T[d], pc)
            nc.vector.tensor_tensor(m1[d], c2q_T[d], S_T[d], op=MUL)
            pq2 = ps.tile([P, LC], F32, tag="a", bufs=4)
            for qi, (qo, qs) in enumerate(QCH):
                nc.tensor.matmul(pq2, tmp[qi][:qs, d * P:(d + 1) * P], P_T[qi][:qs],
                                 start=(qi == 0), stop=(qi == 1))
            nc.vector.tensor_tensor(m2[d], pq2, S_T[d], op=MUL)
        # cqa: out^T[dout, c] = Spart (preloaded) + remaining 18 k-blocks
        xblocks = S_T + c2q_T + m1 + m2
        for mc in range(DC):
            pco = ps.tile([P, LC], F32, tag="a", bufs=4)
            if USE_SPART:
                nc.scalar.copy(pco, Spart[mc])
                for k in range(DC, 4 * DC):
                    nc.tensor.matmul(pco, cqa_WT[k][:, mc * P:(mc + 1) * P],
                                     xblocks[k], start=False, stop=(k == 4 * DC - 1),
                                     skip_group_check=True)
            else:
                for k in range(4 * DC):
                    nc.tensor.matmul(pco, cqa_WT[k][:, mc * P:(mc + 1) * P],
                                     xblocks[k], start=(k == 0), stop=(k == 4 * DC - 1))
            ob = po.tile([P, LC], BF, name=f"ob{tag}{mc}", tag="attb", bufs=2)
            nc.scalar.activation(ob, pco, AF.Identity,
                                 bias=vecs[:, mc, 3:4], scale=1.0)
            nc.sync.dma_start(out=out_t[(row0 + mc) * P:(row0 + mc + 1) * P, :],
                              in_=ob)
            nc.vector.tensor_copy(att_T[row0 + mc], ob)
        return po

    po_q = s2q_call("q", dt['Q_nat'], dt['Q_T'], 0)
    po_q.release()
    po_e = s2q_call("e", dt['E_nat'], dt['E_T'], DC)
    po_e.release()
    cqaw.release(); s2q.release()

    # ---------------- phase 2: knowledge attention stack ----------------
    mp = ctx.enter_context(tc.tile_pool(name="mp", bufs=1))
    wls[1] = alloc_wl(1)
    ke_T = [mp.tile([P, LK], BF, name=f"keT{d}", tag=f"ke{d}", bufs=2) for d in range(DC)]
    for d in range(DC):
        nc.sync.dma_start(out=ke_T[d], in_=dt['KE_T'][d * P:(d + 1) * P, :])

    def mha_ln(x_T, kv_T, wq, wk, wv, wfc, g_ap, b_ap, tag, out_f32=False):
        """x_T: 6 [P,LK] bf16 query-side tiles; kv_T: list of [P,LK] bf16 tiles.
        returns new 6 [P,LK] tiles = LN(fc(attn) + x_T) (bf16, or f32 if out_f32)."""
        nkv = len(kv_T)
        wp = tc.alloc_tile_pool(name=f"wp{tag}", bufs=1)
        # --- projections (streamed weights) ---
        def proj(w_dram, rhs_tiles, nk, out_name, tagbase):
            outs = [mp.tile([P, LK], BF, name=f"{out_name}{m}", tag=f"{tagbase}{m}",
                            bufs=1) for m in range(4)]
            pss = [ps.tile([P, LK], F32, name=f"pss{m}", tag="a", bufs=4) for m in range(4)]
            for k in range(nk):
                wt = wp.tile([P, H * DK], BF, name=f"w{out_name}{k}",
                             tag=f"w{out_name}", bufs=3)
                nc.sync.dma_start(out=wt, in_=w_dram[k * P:(k + 1) * P, :])
                src = rhs_tiles[k]
                for m in range(4):
                    nc.tensor.matmul(pss[m], wt[:, m * P:(m + 1) * P], src,
                                     start=(k == 0), stop=(k == nk - 1))
            for m in range(4):
                nc.vector.tensor_copy(outs[m], pss[m])
            return outs

        q_T = proj(wq, x_T, DC, f"q{tag}", "qT")
        k_T = proj(wk, kv_T, nkv, f"k{tag}", "kT")
        # v in natural layout + ones col: v_aug [LK, 8, 65]
        v_aug = [mp.tile([P, H, DV + 1], BF, name=f"va{tag}{c}", tag=f"va{c}", bufs=1)
                 for c in range(CC)]
        pvs = [ps.tile([P, H * DV], F32, name=f"pvs{m}", tag="a", bufs=4) for m in range(4)]
        for k in range(nkv):
            wt = wp.tile([P, H * DV], BF, name=f"wv{tag}{k}", tag="wv", bufs=3)
            nc.sync.dma_start(out=wt, in_=wv[k * P:(k + 1) * P, :])
            for c in range(CC):
                nc.tensor.matmul(pvs[c], kv_T[k][:, c * P:(c + 1) * P], wt,
                                 start=(k == 0), stop=(k == nkv - 1))
        for c in range(CC):
            nc.vector.tensor_copy(v_aug[c][:, :, 0:DV],
                                  pvs[c].rearrange("p (h d) -> p h d", h=H))
            nc.gpsimd.memset(v_aug[c][:, :, DV:DV + 1], 1.0)
        # --- attention, heads in groups of 4 (batched denominators) ---
        out_T = [mp.tile([P, LK], BF, name=f"o{tag}{m}", tag=f"oT{m}", bufs=1)
                 for m in range(4)]
        for g in range(2):
            povs = []
            for hh in range(4):
                h = g * 4 + hh
                t, o = h // 2, (h % 2) * DK
                e_sb = []
                for c in range(CC):
                    pa = ps.tile([P, LK], F32, tag="a", bufs=4)
                    nc.tensor.matmul(pa, k_T[t][o:o + DK, c * P:(c + 1) * P],
                                     q_T[t][o:o + DK, :], start=True, stop=True)
                    es = mp.tile([P, LK], BF, name=f"es{tag}{h}{c}", tag="es", bufs=8)
                    nc.scalar.activation(es, pa, AF.Exp, scale=SCALE)
                    e_sb.append(es)
                pov = ps.tile([DV + 1, LK], F32, tag="b", bufs=4)
                for c in range(CC):
                    nc.tensor.matmul(pov, v_aug[c][:, h, :], e_sb[c],
                                     start=(c == 0), stop=(c == CC - 1))
                povs.append(pov)
            for hh in range(4):
                h = g * 4 + hh
                t, o = h // 2, (h % 2) * DK
                rr = mp.tile([1, LK], F32, name=f"rr{tag}{h}", tag="rr", bufs=2)
                if USE_APPROX_RECIP:
                    rrs = mp.tile([1, LK], F32, name=f"rrs{tag}{h}", tag="rrs", bufs=2)
                    nc.scalar.copy(rrs, povs[hh][DV:DV + 1, :])
                    nc.vector.reciprocal_approx_fast(out=rr, in_=rrs)
                else:
                    nc.vector.reciprocal(rr, povs[hh][DV:DV + 1, :])
                pbc = mp.tile([DV, LK], F32, name=f"pbc{tag}{h}", tag="pbc", bufs=2)
                if USE_GPS_BCAST:
                    nc.gpsimd.partition_broadcast(pbc, rr)
                else:
                    rr_bf = mp.tile([1, LK], BF, name=f"rrbf{tag}{h}", tag="rrbf", bufs=4)
                    nc.vector.tensor_copy(rr_bf, rr)
                    pbq = ps.tile([DV, LK], F32, tag="a", bufs=4)
                    nc.tensor.matmul(pbq, ones_row[:1, :DV], rr_bf)
                    nc.vector.tensor_copy(pbc, pbq)
                nc.vector.tensor_tensor(out_T[t][o:o + DK, :], povs[hh][:DV, :],
                                        pbc, op=MUL)
        # --- fc + residual + LN ---
        wf = [wp.tile([P, D], BF, name=f"wf{tag}{k}", tag="wf", bufs=4)
              for k in range(4)]
        for k in range(4):
            nc.sync.dma_start(out=wf[k], in_=wfc[k * P:(k + 1) * P, :])
        x1 = [mp.tile([P, LK], BF, name=f"x1{tag}{d}", tag=f"x1{d}", bufs=1)
              for d in range(DC)]
        for d in range(DC):
            pf = ps.tile([P, LK], F32, tag="a", bufs=4)
            for k in range(4):
                nc.tensor.matmul(pf, wf[k][:, d * P:(d + 1) * P], out_T[k],
                                 start=(k == 0), stop=(k == 3))
            nc.vector.tensor_tensor(x1[d], pf, x_T[d], op=ADD)
        # LN stats via ones-matmul over partitions
        ps_s = ps.tile([1, LK], F32, tag="b", bufs=4)
        ps_q = ps.tile([1, LK], F32, tag="b", bufs=4)
        sqs = [mp.tile([P, LK], BF, name=f"sq{tag}{d}", tag="sq", bufs=3)
               for d in range(DC)]
        for d in range(DC):
            nc.vector.tensor_tensor(sqs[d], x1[d], x1[d], op=MUL)
        for d in range(DC):
            nc.tensor.matmul(ps_s, ones_col, x1[d], start=(d == 0), stop=(d == DC - 1))
        for d in range(DC):
            nc.tensor.matmul(ps_q, ones_col, sqs[d], start=(d == 0), stop=(d == DC - 1))
        mu = mp.tile([1, LK], F32, name=f"mu{tag}", tag="mu", bufs=1)
        nc.scalar.activation(mu, ps_s, AF.Copy, bias=0.0, scale=1.0 / D)
        msq = mp.tile([1, LK], F32, name=f"msq{tag}", tag="msq", bufs=1)
        nc.scalar.activation(msq, ps_q, AF.Copy, bias=0.0, scale=1.0 / D)
        var = mp.tile([1, LK], F32, name=f"var{tag}", tag="var", bufs=1)
        nc.vector.tensor_tensor(var, mu, mu, op=MUL)
        nc.vector.tensor_tensor(var, msq, var, op=SUB)
        std = mp.tile([1, LK], F32, name=f"std{tag}", tag="std", bufs=1)
        nc.scalar.activation(std, var, AF.Sqrt, bias=eps_t, scale=1.0)
        warm2 = mp.tile([1, 1], F32, name=f"warm2{tag}", tag="warm", bufs=2)
        nc.scalar.activation(warm2, eps_t, AF.Exp, bias=eps_t, scale=1.0)
        rstd = mp.tile([1, LK], F32, name=f"rstd{tag}", tag="rstd", bufs=1)
        if USE_APPROX_RECIP:
            nc.vector.reciprocal_approx_fast(out=rstd, in_=std)
        else:
            nc.vector.reciprocal(rstd, std)
        c2 = mp.tile([1, LK], F32, name=f"c2{tag}", tag="c2", bufs=1)
        nc.vector.tensor_tensor(c2, mu, rstd, op=MUL)
        pA = mp.tile([P, LK], F32, name=f"pA{tag}", tag="pA", bufs=1)
        pC = mp.tile([P, LK], F32, name=f"pC{tag}", tag="pC", bufs=1)
        if USE_GPS_BCAST:
            nc.gpsimd.partition_broadcast(pA, rstd)
            nc.gpsimd.partition_broadcast(pC, c2)
        else:
            rstd_bf = mp.tile([1, LK], BF, name=f"rstdbf{tag}", tag="rstdbf", bufs=2)
            nc.vector.tensor_copy(rstd_bf, rstd)
            c2_bf = mp.tile([1, LK], BF, name=f"c2bf{tag}", tag="c2bf", bufs=2)
            nc.vector.tensor_copy(c2_bf, c2)
            pAp = ps.tile([P, LK], F32, tag="a", bufs=4)
            nc.tensor.matmul(pAp, ones_row, rstd_bf)
            nc.vector.tensor_copy(pA, pAp)
            pCp = ps.tile([P, LK], F32, tag="a", bufs=4)
            nc.tensor.matmul(pCp, ones_row, c2_bf)
            nc.vector.tensor_copy(pC, pCp)
        y = [mp.tile([P, LK], BF, name=f"y{tag}{d}", tag=f"y{tag[0]}{d}", bufs=1)
             for d in range(DC)]
        for d in range(DC):
            nc.vector.tensor_tensor(y[d], x1[d], pA, op=MUL)
            nc.vector.tensor_tensor(y[d], y[d], pC, op=SUB)
            nc.vector.tensor_scalar(y[d], y[d], g_ap[d], b_ap[d],
                                    op0=MUL, op1=ADD)
        wp.release()
        return y

    cur = ke_T
    for l in range(NL):
        g1 = [lnv[l][:, d, 0:1] for d in range(DC)]
        b1 = [lnv[l][:, d, 1:2] for d in range(DC)]
        g2 = [lnv[l][:, d, 2:3] for d in range(DC)]
        b2 = [lnv[l][:, d, 3:4] for d in range(DC)]
        so = mha_ln(cur, cur, dt[f'sWq{l}'], dt[f'sWk{l}'], dt[f'sWv{l}'],
                    dt[f'sWfc{l}'], g1, b1, f"s{l}")
        cur = mha_ln(so, att_T, dt[f'cWq{l}'], dt[f'cWk{l}'], dt[f'cWv{l}'],
                     dt[f'cWfc{l}'], g2, b2, f"c{l}", out_f32=(l == NL - 1))
    for d in range(DC):
        nc.sync.dma_start(out=out_t[(2 * DC + d) * P:(2 * DC + d + 1) * P, :],
                          in_=cur[d])
    wls[1][0].release()
    ctx.close()


def _t128(a):
    # [n*128, w] -> [128, n*w] so each partition's DMA line is contiguous
    n = a.shape[0] // P
    return np.ascontiguousarray(
        a.reshape(n, P, a.shape[1]).transpose(1, 0, 2).reshape(P, -1))


def _t128pad(a):
    # ragged rows (LQ=160): pad to 2*128 rows then tile
    out = np.zeros((2 * P, a.shape[1]), a.dtype)
    out[:a.shape[0]] = a
    return _t128(out)


def kernel(**inputs):
    if 'nc' not in _CACHE:
        _CACHE['nc'] = _build()
    nc = _CACHE['nc']
    f = lambda x: np.ascontiguousarray(np.asarray(x), dtype=np.float32)
    bf = lambda x: np.asarray(x, dtype=np.float32).astype(NPBF)
    bfT = lambda x: np.asarray(x, dtype=np.float32).T.astype(NPBF)
    seq = f(inputs['sequences']); qry = f(inputs['query']); evd = f(inputs['evidence'])
    ke = f(inputs['knowledge_embed'])
    vecs = _t128(np.ascontiguousarray(np.stack(
        [f(inputs['w4C'])[:, 0], f(inputs['w4Q'])[:, 0],
         f(inputs['w4mlu'])[0, 0, :], f(inputs['cqa_b'])], axis=1)))
    cqa_WT = _t128(bfT(inputs['cqa_W']))
    lwb = {}
    for l in range(NL):
        lwb[f'sWq{l}'] = _t128(bf(inputs['L_sWq'][l]))
        lwb[f'sWk{l}'] = _t128(bf(inputs['L_sWk'][l]))
        lwb[f'sWv{l}'] = _t128(bf(inputs['L_sWv'][l]))
        lwb[f'sWfc{l}'] = _t128(bf(inputs['L_sWfc'][l]))
        lwb[f'cWq{l}'] = _t128(bf(inputs['L_cWq'][l]))
        lwb[f'cWk{l}'] = _t128(bf(inputs['L_cWk'][l]))
        lwb[f'cWv{l}'] = _t128(bf(inputs['L_cWv'][l]))
        lwb[f'cWfc{l}'] = _t128(bf(inputs['L_cWfc'][l]))
        lwb[f'ln{l}'] = _t128(np.ascontiguousarray(np.stack(
            [f(inputs['L_n1g'][l]), f(inputs['L_n1b'][l]),
             f(inputs['L_n2g'][l]), f(inputs['L_n2b'][l])], axis=1)))
    in_maps = []
    for b in range(B):
        m = {
            'S_nat': _t128(bf(seq[b])), 'S_T': _t128(bfT(seq[b])),
            'Q_nat': _t128pad(bf(qry[b])), 'Q_T': _t128(bfT(qry[b])),
            'E_nat': _t128pad(bf(evd[b])), 'E_T': _t128(bfT(evd[b])),
            'KE_T': _t128(bfT(ke[b])),
            'vecs': vecs, 'cqa_WT': cqa_WT,
        }
        m.update(lwb)
        in_maps.append(m)
    _CACHE['last_in_maps'] = in_maps
    res = run_bass_kernel_spmd(nc, in_maps, core_ids=list(range(B)))
    _CACHE['last_results'] = res
    outs = np.stack([np.asarray(r['out_t'], dtype=np.float32)
                     for r in res.results])                      # [B, 2304, 512]
    out = np.concatenate([seq, outs.transpose(0, 2, 1)], axis=-1)
    return out
